# revision 47
# baseline (speedup 1.0000x reference)
"""Chamfer distance (pytorch3d defaults) on 8 Trainium2 NeuronCores.

Problem: gts_X, pred_X: [4, 8192, 3] fp32. loss = mean_b mean_n min_p d(x_bn, y_bp)
                                              + mean_b mean_p min_n d(x_bn, y_bp),
d = squared euclidean distance. gts_normals is unused (reference default path).

Sharding: 8 independent tasks = 4 batches x 2 directions, one per core.
Each core computes per-query min_r d(Q_q, R_r) for its (Q, R) pair of
8192-point clouds; the host sums, guards, and averages.

Device algorithm per core:
- Queries are split into 64 spatially-compact leaves of 128 (k-d median
  splits on the widest dim). For each leaf the host gathers the CAND=224
  refs nearest to the leaf's bounding box; the device scans only those.
  Exactness guard: every EXCLUDED ref is at least T from the leaf box, so
  for a query at distance d_in inside the box, any excluded ref is at
  least T + d_in away (the segment to it crosses the box boundary). The
  host flags queries whose found min exceeds (T + d_in)^2 (with rounding
  margin) and recomputes them exactly in numpy, so the result is exact
  for any input.
- Per (128q x 224r) tile ONE K=13 bf16 matmul computes the full
  |Q|^2 + |R|^2 - 2 Q.R via an exact hi/lo bf16 split (bf16 products are
  exact in fp32, PSUM accumulates fp32; the negligible lo*lo cross term
  is dropped).
- Matmuls are packed 4x with tile_position row groups into one 4-bank
  PSUM tile; each row-group replica only holds the leaves it processes.
  lhs and rhs share one dram tensor split into an early chunk (lhs + 2
  groups of cands) and a late chunk so the first groups' matmuls only
  wait on the early DMAs, spread over the sync/scalar/gpsimd queues.
- Drain: middle groups are copied PSUM->SBUF with a bf16 downcast by ONE
  ACT op per group (4 leaves, saves per-op overhead), then folded by a
  DVE tensor_tensor min tree in 2x bf16 mode. The first and last groups
  instead drain 3 leaves via ACT + 1 leaf min-reduced directly from PSUM
  by the DVE (fp32), which hides the ACT table load at the start and
  balances the engines at the edges.
"""

import sys

sys.path.insert(0, "/opt/trn_rl_repo")

import numpy as np
import ml_dtypes

import concourse.bacc as bacc
import concourse.mybir as mybir
from concourse.tile import TileContext
from concourse.bass_utils import run_bass_kernel_spmd

BF16 = ml_dtypes.bfloat16

B = 4
N = 8192
K = 13  # QhRh(3) + QhRl(3) + QlRh(3) + |Q|^2 hi/lo (2) + |R|^2 hi/lo (2)
MBLK = 128  # queries per leaf (PSUM partitions)
CAND = 224  # gathered candidate refs per leaf
NMB = N // MBLK  # 64 leaves
NG = NMB // 4  # 16 groups of 4 row-group-packed leaves
GA = 4  # groups whose candidates ride in the early chunk
LCOLS = MBLK * NG  # 2048 lhs cols per replica
RCOLS = CAND * NG  # rhs cols per replica
ACOLS = LCOLS + GA * CAND  # early chunk: lhs + first GA groups of cands
BCOLS = (NG - GA) * CAND  # late chunk

LAST_RESULTS = None  # BassKernelResults of the most recent run (for test.py)


def _build_bass():
    nc = bacc.Bacc("TRN2")
    # replica j (rows 13j..13j+12) serves leaves m = 4g+j:
    #   cols 0:2048 = queries (128 per group), cols 2048+320g.. = candidates
    inp = nc.dram_tensor("inp", [4 * K, LCOLS + RCOLS], mybir.dt.bfloat16,
                         kind="ExternalInput")
    out = nc.dram_tensor("out", [MBLK, NMB], mybir.dt.float32, kind="ExternalOutput")

    with TileContext(nc) as tc:
        with (
            tc.tile_pool(name="data", bufs=1) as data_pool,
            tc.tile_pool(name="work", bufs=3) as work_pool,
            tc.tile_pool(name="ps", bufs=2, space="PSUM") as ps_pool,
        ):
            # two SBUF tiles so the first groups' matmuls only depend on
            # the early-chunk DMAs (tile deps are conservative per tile)
            sbA = data_pool.tile([128, ACOLS], mybir.dt.bfloat16)
            sbB = data_pool.tile([128, BCOLS], mybir.dt.bfloat16)

            # early chunks (lhs + GA groups) spread over the 3 DMA-capable
            # queues; late chunks stay off the ACT queue
            for eng, j in ((nc.sync, 0), (nc.scalar, 1), (nc.gpsimd, 2),
                           (nc.sync, 3)):
                eng.dma_start(sbA[32 * j : 32 * j + K, :],
                              inp.ap()[K * j : K * j + K, 0:ACOLS])
            for eng, j in ((nc.gpsimd, 0), (nc.gpsimd, 1), (nc.sync, 2),
                           (nc.sync, 3)):
                eng.dma_start(sbB[32 * j : 32 * j + K, :],
                              inp.ap()[K * j : K * j + K, ACOLS:])

            blockmins = data_pool.tile([MBLK, NMB], mybir.dt.float32)

            def direct(ps, g, j):
                nc.vector.tensor_reduce(
                    blockmins[:, 4 * g + j : 4 * g + j + 1],
                    ps[:, j, 0:CAND],
                    axis=mybir.AxisListType.X,
                    op=mybir.AluOpType.min,
                )

            prev = None  # (bfb tile, group idx, nblocks) pending fold
            for g in range(NG):
                ps = ps_pool.tile([MBLK, 4, 512], mybir.dt.float32, tag="ps")
                if g < GA:
                    rhs_ap = sbA
                    rc = LCOLS + g * CAND
                else:
                    rhs_ap = sbB
                    rc = (g - GA) * CAND
                for j in range(4):
                    nc.tensor.matmul(
                        ps[:, j, 0:CAND],
                        sbA[32 * j : 32 * j + K,
                            g * MBLK : (g + 1) * MBLK],
                        rhs_ap[32 * j : 32 * j + K, rc : rc + CAND],
                        start=True,
                        stop=True,
                        tile_position=(32 * j, 0),
                    )
                if g == 0 or g == NG - 1:
                    # edge groups: lighter DVE load (1 direct + 3-leaf fold)
                    # so the DVE doesn't lag the ACT cadence at the edges
                    direct(ps, g, 3)
                    bfb = work_pool.tile([MBLK, 3, CAND], mybir.dt.bfloat16,
                                         tag="bfe")
                    nc.scalar.copy(bfb[:], ps[:, 0:3, 0:CAND])
                    if prev is not None:
                        _fold(nc, work_pool, blockmins, *prev)
                    if g == NG - 1:
                        _fold(nc, work_pool, blockmins, bfb, g, 3)
                    else:
                        prev = (bfb, g, 3)
                else:
                    bfb = work_pool.tile([MBLK, 4, CAND], mybir.dt.bfloat16,
                                         tag="bfb")
                    nc.scalar.copy(bfb[:], ps[:, :, 0:CAND])
                    if prev is not None:
                        _fold(nc, work_pool, blockmins, *prev)
                    prev = (bfb, g, 4)

            nc.sync.dma_start(out.ap(), blockmins[:])
    return nc


def _fold(nc, work_pool, blockmins, bfb, g, nb):
    """Fold bfb [128, nb, 320] bf16 to blockmins[:, 4g:4g+nb] via DVE
    TT-min levels (2x bf16 mode) plus a final short reduce."""
    h = CAND // 2
    t1 = work_pool.tile([MBLK, nb, h], mybir.dt.bfloat16, tag=f"t1{nb}")
    t2 = work_pool.tile([MBLK, nb, h // 2], mybir.dt.bfloat16, tag=f"t2{nb}")
    t3 = work_pool.tile([MBLK, nb, h // 4], mybir.dt.bfloat16, tag=f"t3{nb}")
    nc.vector.tensor_tensor(t1[:], bfb[:, :, 0:h], bfb[:, :, h : 2 * h],
                            op=mybir.AluOpType.min)
    nc.vector.tensor_tensor(t2[:], t1[:, :, 0 : h // 2], t1[:, :, h // 2 : h],
                            op=mybir.AluOpType.min)
    nc.vector.tensor_tensor(t3[:], t2[:, :, 0 : h // 4], t2[:, :, h // 4 : h // 2],
                            op=mybir.AluOpType.min)
    nc.vector.tensor_reduce(
        blockmins[:, 4 * g : 4 * g + nb],
        t3[:],
        axis=mybir.AxisListType.X,
        op=mybir.AluOpType.min,
    )


def _split_bf16(v):
    """v (fp32) ~= hi + lo with both bf16; residual is O(2^-18 |v|)."""
    hi = v.astype(BF16)
    lo = (v - hi.astype(np.float32)).astype(BF16)
    return hi, lo


def _kd_leaves(P):
    """Split points into 64 leaves of 128 by recursive widest-dim median
    splits. Returns list of index arrays in leaf order."""
    out = []

    def rec(ix):
        if len(ix) <= MBLK:
            out.append(ix)
            return
        Pi = P[ix]
        dim = int(np.argmax(Pi.max(0) - Pi.min(0)))
        half = len(ix) // 2
        ordr = np.argpartition(Pi[:, dim], half)
        rec(ix[ordr[:half]])
        rec(ix[ordr[half:]])

    rec(np.arange(len(P)))
    return out


def _prep_core(Q0, R0):
    """Build device inputs for one (queries, refs) pair.

    Returns (in_map, post) where post carries what the host needs to
    finish: permuted queries, per-query guard thresholds, refs.
    """
    leaves = _kd_leaves(Q0)
    perm = np.concatenate(leaves)
    Qs = Q0[perm]  # rank r = 128*m + p

    R64 = R0.astype(np.float64)
    cands = np.empty((NMB, CAND), np.int64)
    guard = np.empty(N)  # per rank: (T + d_in)^2
    for m, ix in enumerate(leaves):
        q = Q0[ix]
        lo, hi = q.min(0), q.max(0)
        dbox2 = (np.maximum(np.maximum(lo - R64, R64 - hi), 0.0) ** 2).sum(1)
        ordr = np.argpartition(dbox2, CAND)
        cands[m] = ordr[:CAND]
        T = np.sqrt(dbox2[ordr[CAND:]].min())
        d_in = np.minimum(q - lo, hi - q).min(1)
        guard[m * MBLK : (m + 1) * MBLK] = (T + np.maximum(d_in, 0.0)) ** 2

    Qh, Ql = _split_bf16(Qs)  # [N, 3]
    nQh, nQl = _split_bf16((Qs * Qs).sum(axis=1))
    Rg = R0[cands.reshape(-1)]  # [NMB*CAND, 3] gathered refs
    Rh, Rl = _split_bf16(-2.0 * Rg)
    nRh, nRl = _split_bf16((Rg * Rg).sum(axis=1))
    one = np.ones((), dtype=BF16)

    # replica j serves leaves m = 4g+j
    inp = np.empty([4 * K, LCOLS + RCOLS], dtype=BF16)
    for j in range(4):
        qsel = (
            (np.arange(NG)[:, None] * 4 + j) * MBLK + np.arange(MBLK)[None, :]
        ).reshape(-1)
        rsel = (
            (np.arange(NG)[:, None] * 4 + j) * CAND + np.arange(CAND)[None, :]
        ).reshape(-1)
        L = inp[:, 0:LCOLS]
        Rm = inp[:, LCOLS:]
        L[K * j + 0 : K * j + 3] = Qh[qsel].T
        L[K * j + 3 : K * j + 6] = Qh[qsel].T
        L[K * j + 6 : K * j + 9] = Ql[qsel].T
        L[K * j + 9] = nQh[qsel]
        L[K * j + 10] = nQl[qsel]
        L[K * j + 11 : K * j + 13] = one
        Rm[K * j + 0 : K * j + 3] = Rh[rsel].T
        Rm[K * j + 3 : K * j + 6] = Rl[rsel].T
        Rm[K * j + 6 : K * j + 9] = Rh[rsel].T
        Rm[K * j + 9 : K * j + 11] = one
        Rm[K * j + 11] = nRh[rsel]
        Rm[K * j + 12] = nRl[rsel]

    in_map = {"inp": inp}
    post = (Qs, guard, R64)
    return in_map, post


def _finish_core(dev_out, post):
    """Host: apply the exactness guard and recompute flagged queries
    exactly. Returns per-query min sum."""
    Qs, guard, R64 = post
    mins = dev_out.astype(np.float64).T.reshape(-1)  # rank-ordered
    # margin for bf16 downcast (~2^-9 rel) and dropped lo*lo term (~4e-5 abs)
    thr = guard * (1.0 - 1e-2) - 1e-3
    bad = np.nonzero(mins > thr)[0]
    if len(bad):
        Qb = Qs[bad].astype(np.float64)
        d = ((Qb[:, None, :] - R64[None, :, :]) ** 2).sum(-1)
        mins[bad] = d.min(axis=1)
    return mins.sum()


def _try_axon_reset():
    """The axon-tunneled device sporadically wedges (NRT_EXEC_UNIT_UNRECOVERABLE);
    axon_reset() recovers it."""
    try:
        import ctypes

        import jax

        jax.devices()
        lib = ctypes.CDLL("/opt/axon/libaxon_pjrt.so")
        lib.axon_reset.restype = ctypes.c_int64
        lib.axon_reset()
    except Exception:
        pass


def _task_pairs(gts_X, pred_X):
    for b in range(B):
        yield gts_X[b], pred_X[b]  # each gts point -> nearest pred
        yield pred_X[b], gts_X[b]  # each pred point -> nearest gts


def kernel(gts_X, pred_X, gts_normals=None, **_ignored):
    global LAST_RESULTS
    gts_X = np.asarray(gts_X, dtype=np.float32)
    pred_X = np.asarray(pred_X, dtype=np.float32)
    assert gts_X.shape == (B, N, 3) and pred_X.shape == (B, N, 3)

    in_maps = []
    posts = []
    for Qr, Rr in _task_pairs(gts_X, pred_X):
        in_map, post = _prep_core(Qr, Rr)
        in_maps.append(in_map)
        posts.append(post)

    nc = _build_bass()
    nc.finalize()
    res = None
    for attempt in range(3):
        try:
            res = run_bass_kernel_spmd(nc, in_maps, core_ids=list(range(8)))
            break
        except Exception:
            if attempt == 2:
                raise
            _try_axon_reset()
    LAST_RESULTS = res

    total = 0.0
    for post, r in zip(posts, res.results):
        total += _finish_core(r["out"], post)

    loss = total / (B * N)
    return np.asarray(loss, dtype=np.float32)


# revision 48
# speedup vs baseline: 1.0074x; 1.0074x over previous
"""Chamfer distance (pytorch3d defaults) on 8 Trainium2 NeuronCores.

Problem: gts_X, pred_X: [4, 8192, 3] fp32. loss = mean_b mean_n min_p d(x_bn, y_bp)
                                              + mean_b mean_p min_n d(x_bn, y_bp),
d = squared euclidean distance. gts_normals is unused (reference default path).

Sharding: 8 independent tasks = 4 batches x 2 directions, one per core.
Each core computes per-query min_r d(Q_q, R_r) for its (Q, R) pair of
8192-point clouds; the host sums, guards, and averages.

Device algorithm per core:
- Queries are split into 64 spatially-compact leaves of 128 (k-d median
  splits on the widest dim). For each leaf the host gathers the CAND=224
  refs nearest to the leaf's bounding box; the device scans only those.
  Exactness guard: every EXCLUDED ref is at least T from the leaf box, so
  for a query at distance d_in inside the box, any excluded ref is at
  least T + d_in away (the segment to it crosses the box boundary). The
  host flags queries whose found min exceeds (T + d_in)^2 (with rounding
  margin) and recomputes them exactly in numpy, so the result is exact
  for any input.
- Per (128q x 224r) tile ONE K=13 bf16 matmul computes the full
  |Q|^2 + |R|^2 - 2 Q.R via an exact hi/lo bf16 split (bf16 products are
  exact in fp32, PSUM accumulates fp32; the negligible lo*lo cross term
  is dropped).
- Matmuls are packed 4x with tile_position row groups into one 4-bank
  PSUM tile; each row-group replica only holds the leaves it processes.
  lhs and rhs share one dram tensor split into an early chunk (lhs + 2
  groups of cands) and a late chunk so the first groups' matmuls only
  wait on the early DMAs, spread over the sync/scalar/gpsimd queues.
- Drain: middle groups are copied PSUM->SBUF with a bf16 downcast by ONE
  ACT op per group (4 leaves, saves per-op overhead), then folded by a
  DVE tensor_tensor min tree in 2x bf16 mode. The first and last groups
  instead drain 3 leaves via ACT + 1 leaf min-reduced directly from PSUM
  by the DVE (fp32), which hides the ACT table load at the start and
  balances the engines at the edges.
"""

import sys

sys.path.insert(0, "/opt/trn_rl_repo")

import numpy as np
import ml_dtypes

import concourse.bacc as bacc
import concourse.mybir as mybir
from concourse.tile import TileContext
from concourse.bass_utils import run_bass_kernel_spmd

BF16 = ml_dtypes.bfloat16

B = 4
N = 8192
K = 13  # QhRh(3) + QhRl(3) + QlRh(3) + |Q|^2 hi/lo (2) + |R|^2 hi/lo (2)
MBLK = 128  # queries per leaf (PSUM partitions)
CAND = 224  # gathered candidate refs per leaf
NMB = N // MBLK  # 64 leaves
NG = NMB // 4  # 16 groups of 4 row-group-packed leaves
GA = 2  # groups whose candidates ride in the early chunk
LCOLS = MBLK * NG  # 2048 lhs cols per replica
RCOLS = CAND * NG  # rhs cols per replica
ACOLS = LCOLS + GA * CAND  # early chunk: lhs + first GA groups of cands
BCOLS = (NG - GA) * CAND  # late chunk

LAST_RESULTS = None  # BassKernelResults of the most recent run (for test.py)


def _build_bass():
    nc = bacc.Bacc("TRN2")
    # replica j (rows 13j..13j+12) serves leaves m = 4g+j:
    #   cols 0:2048 = queries (128 per group), cols 2048+320g.. = candidates
    inp = nc.dram_tensor("inp", [4 * K, LCOLS + RCOLS], mybir.dt.bfloat16,
                         kind="ExternalInput")
    out = nc.dram_tensor("out", [MBLK, NMB], mybir.dt.float32, kind="ExternalOutput")

    with TileContext(nc) as tc:
        with (
            tc.tile_pool(name="data", bufs=1) as data_pool,
            tc.tile_pool(name="work", bufs=3) as work_pool,
            tc.tile_pool(name="ps", bufs=2, space="PSUM") as ps_pool,
        ):
            # two SBUF tiles so the first groups' matmuls only depend on
            # the early-chunk DMAs (tile deps are conservative per tile)
            sbA = data_pool.tile([128, ACOLS], mybir.dt.bfloat16)
            sbB = data_pool.tile([128, BCOLS], mybir.dt.bfloat16)

            # early chunks (lhs + GA groups) spread over the 3 DMA-capable
            # queues; late chunks stay off the ACT queue
            for eng, j in ((nc.sync, 0), (nc.scalar, 1), (nc.gpsimd, 2),
                           (nc.sync, 3)):
                eng.dma_start(sbA[32 * j : 32 * j + K, :],
                              inp.ap()[K * j : K * j + K, 0:ACOLS])
            for eng, j in ((nc.gpsimd, 0), (nc.gpsimd, 1), (nc.sync, 2),
                           (nc.sync, 3)):
                eng.dma_start(sbB[32 * j : 32 * j + K, :],
                              inp.ap()[K * j : K * j + K, ACOLS:])

            blockmins = data_pool.tile([MBLK, NMB], mybir.dt.float32)

            def direct(ps, g, j):
                nc.vector.tensor_reduce(
                    blockmins[:, 4 * g + j : 4 * g + j + 1],
                    ps[:, j, 0:CAND],
                    axis=mybir.AxisListType.X,
                    op=mybir.AluOpType.min,
                )

            prev = None  # (bfb tile, group idx, nblocks) pending fold
            for g in range(NG):
                ps = ps_pool.tile([MBLK, 4, 512], mybir.dt.float32, tag="ps")
                if g < GA:
                    rhs_ap = sbA
                    rc = LCOLS + g * CAND
                else:
                    rhs_ap = sbB
                    rc = (g - GA) * CAND
                for j in range(4):
                    nc.tensor.matmul(
                        ps[:, j, 0:CAND],
                        sbA[32 * j : 32 * j + K,
                            g * MBLK : (g + 1) * MBLK],
                        rhs_ap[32 * j : 32 * j + K, rc : rc + CAND],
                        start=True,
                        stop=True,
                        tile_position=(32 * j, 0),
                    )
                if g == 0 or g == NG - 1:
                    # edge groups: lighter DVE load (1 direct + 3-leaf fold)
                    # so the DVE doesn't lag the ACT cadence at the edges
                    direct(ps, g, 3)
                    bfb = work_pool.tile([MBLK, 3, CAND], mybir.dt.bfloat16,
                                         tag="bfe")
                    nc.scalar.copy(bfb[:], ps[:, 0:3, 0:CAND])
                    if prev is not None:
                        _fold(nc, work_pool, blockmins, *prev)
                    if g == NG - 1:
                        _fold(nc, work_pool, blockmins, bfb, g, 3)
                    else:
                        prev = (bfb, g, 3)
                else:
                    bfb = work_pool.tile([MBLK, 4, CAND], mybir.dt.bfloat16,
                                         tag="bfb")
                    nc.scalar.copy(bfb[:], ps[:, :, 0:CAND])
                    if prev is not None:
                        _fold(nc, work_pool, blockmins, *prev)
                    prev = (bfb, g, 4)

            nc.sync.dma_start(out.ap(), blockmins[:])
    return nc


def _fold(nc, work_pool, blockmins, bfb, g, nb):
    """Fold bfb [128, nb, 320] bf16 to blockmins[:, 4g:4g+nb] via DVE
    TT-min levels (2x bf16 mode) plus a final short reduce."""
    h = CAND // 2
    t1 = work_pool.tile([MBLK, nb, h], mybir.dt.bfloat16, tag=f"t1{nb}")
    t2 = work_pool.tile([MBLK, nb, h // 2], mybir.dt.bfloat16, tag=f"t2{nb}")
    t3 = work_pool.tile([MBLK, nb, h // 4], mybir.dt.bfloat16, tag=f"t3{nb}")
    nc.vector.tensor_tensor(t1[:], bfb[:, :, 0:h], bfb[:, :, h : 2 * h],
                            op=mybir.AluOpType.min)
    nc.vector.tensor_tensor(t2[:], t1[:, :, 0 : h // 2], t1[:, :, h // 2 : h],
                            op=mybir.AluOpType.min)
    nc.vector.tensor_tensor(t3[:], t2[:, :, 0 : h // 4], t2[:, :, h // 4 : h // 2],
                            op=mybir.AluOpType.min)
    nc.vector.tensor_reduce(
        blockmins[:, 4 * g : 4 * g + nb],
        t3[:],
        axis=mybir.AxisListType.X,
        op=mybir.AluOpType.min,
    )


def _split_bf16(v):
    """v (fp32) ~= hi + lo with both bf16; residual is O(2^-18 |v|)."""
    hi = v.astype(BF16)
    lo = (v - hi.astype(np.float32)).astype(BF16)
    return hi, lo


def _kd_leaves(P):
    """Split points into 64 leaves of 128 by recursive widest-dim median
    splits. Returns list of index arrays in leaf order."""
    out = []

    def rec(ix):
        if len(ix) <= MBLK:
            out.append(ix)
            return
        Pi = P[ix]
        dim = int(np.argmax(Pi.max(0) - Pi.min(0)))
        half = len(ix) // 2
        ordr = np.argpartition(Pi[:, dim], half)
        rec(ix[ordr[:half]])
        rec(ix[ordr[half:]])

    rec(np.arange(len(P)))
    return out


def _prep_core(Q0, R0):
    """Build device inputs for one (queries, refs) pair.

    Returns (in_map, post) where post carries what the host needs to
    finish: permuted queries, per-query guard thresholds, refs.
    """
    leaves = _kd_leaves(Q0)
    perm = np.concatenate(leaves)
    Qs = Q0[perm]  # rank r = 128*m + p

    R64 = R0.astype(np.float64)
    cands = np.empty((NMB, CAND), np.int64)
    guard = np.empty(N)  # per rank: (T + d_in)^2
    for m, ix in enumerate(leaves):
        q = Q0[ix]
        lo, hi = q.min(0), q.max(0)
        dbox2 = (np.maximum(np.maximum(lo - R64, R64 - hi), 0.0) ** 2).sum(1)
        ordr = np.argpartition(dbox2, CAND)
        cands[m] = ordr[:CAND]
        T = np.sqrt(dbox2[ordr[CAND:]].min())
        d_in = np.minimum(q - lo, hi - q).min(1)
        guard[m * MBLK : (m + 1) * MBLK] = (T + np.maximum(d_in, 0.0)) ** 2

    Qh, Ql = _split_bf16(Qs)  # [N, 3]
    nQh, nQl = _split_bf16((Qs * Qs).sum(axis=1))
    Rg = R0[cands.reshape(-1)]  # [NMB*CAND, 3] gathered refs
    Rh, Rl = _split_bf16(-2.0 * Rg)
    nRh, nRl = _split_bf16((Rg * Rg).sum(axis=1))
    one = np.ones((), dtype=BF16)

    # replica j serves leaves m = 4g+j
    inp = np.empty([4 * K, LCOLS + RCOLS], dtype=BF16)
    for j in range(4):
        qsel = (
            (np.arange(NG)[:, None] * 4 + j) * MBLK + np.arange(MBLK)[None, :]
        ).reshape(-1)
        rsel = (
            (np.arange(NG)[:, None] * 4 + j) * CAND + np.arange(CAND)[None, :]
        ).reshape(-1)
        L = inp[:, 0:LCOLS]
        Rm = inp[:, LCOLS:]
        L[K * j + 0 : K * j + 3] = Qh[qsel].T
        L[K * j + 3 : K * j + 6] = Qh[qsel].T
        L[K * j + 6 : K * j + 9] = Ql[qsel].T
        L[K * j + 9] = nQh[qsel]
        L[K * j + 10] = nQl[qsel]
        L[K * j + 11 : K * j + 13] = one
        Rm[K * j + 0 : K * j + 3] = Rh[rsel].T
        Rm[K * j + 3 : K * j + 6] = Rl[rsel].T
        Rm[K * j + 6 : K * j + 9] = Rh[rsel].T
        Rm[K * j + 9 : K * j + 11] = one
        Rm[K * j + 11] = nRh[rsel]
        Rm[K * j + 12] = nRl[rsel]

    in_map = {"inp": inp}
    post = (Qs, guard, R64)
    return in_map, post


def _finish_core(dev_out, post):
    """Host: apply the exactness guard and recompute flagged queries
    exactly. Returns per-query min sum."""
    Qs, guard, R64 = post
    mins = dev_out.astype(np.float64).T.reshape(-1)  # rank-ordered
    # margin for bf16 downcast (~2^-9 rel) and dropped lo*lo term (~4e-5 abs)
    thr = guard * (1.0 - 1e-2) - 1e-3
    bad = np.nonzero(mins > thr)[0]
    if len(bad):
        Qb = Qs[bad].astype(np.float64)
        d = ((Qb[:, None, :] - R64[None, :, :]) ** 2).sum(-1)
        mins[bad] = d.min(axis=1)
    return mins.sum()


def _try_axon_reset():
    """The axon-tunneled device sporadically wedges (NRT_EXEC_UNIT_UNRECOVERABLE);
    axon_reset() recovers it."""
    try:
        import ctypes

        import jax

        jax.devices()
        lib = ctypes.CDLL("/opt/axon/libaxon_pjrt.so")
        lib.axon_reset.restype = ctypes.c_int64
        lib.axon_reset()
    except Exception:
        pass


def _task_pairs(gts_X, pred_X):
    for b in range(B):
        yield gts_X[b], pred_X[b]  # each gts point -> nearest pred
        yield pred_X[b], gts_X[b]  # each pred point -> nearest gts


def kernel(gts_X, pred_X, gts_normals=None, **_ignored):
    global LAST_RESULTS
    gts_X = np.asarray(gts_X, dtype=np.float32)
    pred_X = np.asarray(pred_X, dtype=np.float32)
    assert gts_X.shape == (B, N, 3) and pred_X.shape == (B, N, 3)

    in_maps = []
    posts = []
    for Qr, Rr in _task_pairs(gts_X, pred_X):
        in_map, post = _prep_core(Qr, Rr)
        in_maps.append(in_map)
        posts.append(post)

    nc = _build_bass()
    nc.finalize()
    res = None
    for attempt in range(3):
        try:
            res = run_bass_kernel_spmd(nc, in_maps, core_ids=list(range(8)))
            break
        except Exception:
            if attempt == 2:
                raise
            _try_axon_reset()
    LAST_RESULTS = res

    total = 0.0
    for post, r in zip(posts, res.results):
        total += _finish_core(r["out"], post)

    loss = total / (B * N)
    return np.asarray(loss, dtype=np.float32)
